# revision 2
# baseline (speedup 1.0000x reference)
"""Causal self-attention (B=2, T=2048, C=768, H=12) on 8 TRN2 NeuronCores.

Sharding: core c = (b = c // 4, head-group hg = c % 4 of 3 heads).
Each core: QKV projection for its 3 heads (column-parallel), causal
attention, and a row-parallel slice of the output projection. The host
pre-transposes/casts operands to bf16 and sums the 4 partial outputs
per batch (row-parallel all-reduce done host-side) + bias.

Attention uses the Y^T = V^T @ P formulation: scores are computed
transposed [k, q], exp'd, and one wide matmul per (j-block, head)
accumulates Y^T [d, q] in PSUM with the softmax denominator riding in
an extra ones column of V. Normalization = reciprocal of the denom row,
broadcast via a 1-contraction fp16 matmul, multiplied on DVE.
"""

import os
import sys

import numpy as np
import ml_dtypes


def _ensure_paths():
    for p in ("/opt/trn_rl_repo", "/opt/pypackages"):
        if os.path.isdir(p) and p not in sys.path:
            sys.path.append(p)


_ensure_paths()

import concourse.bass as bass  # noqa: E402
import concourse.mybir as mybir  # noqa: E402
import concourse.tile as tile  # noqa: E402
from concourse import bacc  # noqa: E402
from concourse.bass_utils import run_bass_kernel_spmd  # noqa: E402

BF16 = ml_dtypes.bfloat16
F16 = np.float16

B, T, C, H, D = 2, 2048, 768, 12, 64
G = 3                # heads per core
FQK = 512            # q(192) | pad(64) | k(192) | pad(64) -> q/k same partition offsets
FV = G * (D + 1)     # 195: per head [d0..d63 | ones]
NT = T // 128        # 16 token tiles
KS = C // 128        # 6 contraction subtiles
QC = NT // 4         # 4 q-chunks of 512

_cache: dict[bool, object] = {}
_last_in_maps = None


def _build(causal: bool):
    dt = mybir.dt
    nc = bacc.Bacc("TRN2", num_devices=8)

    xT_d = nc.dram_tensor("xT", [C, T], dt.bfloat16, kind="ExternalInput")
    wqkT_d = nc.dram_tensor("wqkT", [C, FQK], dt.bfloat16, kind="ExternalInput")
    wvT_d = nc.dram_tensor("wvT", [C, FV], dt.bfloat16, kind="ExternalInput")
    bqk_d = nc.dram_tensor("bqk", [128, 4], dt.float32, kind="ExternalInput")
    bv_d = nc.dram_tensor("bv", [128, FV], dt.float32, kind="ExternalInput")
    wpT_d = nc.dram_tensor("wpT", [256, C], dt.bfloat16, kind="ExternalInput")
    maskT_d = nc.dram_tensor("maskT", [128, 128], dt.bfloat16, kind="ExternalInput")
    out_d = nc.dram_tensor("out", [T, C], dt.float16, kind="ExternalOutput")

    Exp = mybir.ActivationFunctionType.Exp

    with tile.TileContext(nc) as tc:
        with tc.tile_pool(name="persist", bufs=1) as pp:
            xT_sb = pp.tile([128, KS, T], dt.bfloat16)
            wqkT_sb = pp.tile([128, KS, FQK], dt.bfloat16)
            wvT_sb = pp.tile([128, KS, FV], dt.bfloat16)
            wpT_sb = pp.tile([128, 2, C], dt.bfloat16)
            bqk_sb = pp.tile([128, 4], dt.float32)
            bv_sb = pp.tile([128, FV], dt.float32)
            maskT_sb = pp.tile([128, 128], dt.bfloat16)
            ones16 = pp.tile([128, 64], dt.float16)
            qkT_sb = pp.tile([128, 4, T], dt.bfloat16)
            v_sb = pp.tile([128, NT, FV], dt.bfloat16)
            yT_sb = pp.tile([128, 2, T], dt.bfloat16)

            # input DMAs issued from the Pool queue (cheap issue slot)
            nc.gpsimd.dma_start(maskT_sb[:], maskT_d.ap())
            nc.gpsimd.dma_start(bqk_sb[:], bqk_d.ap())
            nc.gpsimd.dma_start(bv_sb[:], bv_d.ap())
            for s in range(KS):
                nc.gpsimd.dma_start(
                    wqkT_sb[:, s, :], wqkT_d.ap()[s * 128 : (s + 1) * 128, :]
                )
                nc.gpsimd.dma_start(
                    xT_sb[:, s, :], xT_d.ap()[s * 128 : (s + 1) * 128, :]
                )
            nc.gpsimd.dma_start(
                wvT_sb[:], wvT_d.ap().rearrange("(s p) f -> p s f", p=128)
            )
            nc.gpsimd.dma_start(
                wpT_sb[:], wpT_d.ap().rearrange("(s p) o -> p s o", p=128)
            )
            nc.gpsimd.memset(ones16[:], 1.0)

            # ---- Phase 1a: q/k projection -> qkT_sb [f, t] (bf16, +bias) ----
            with tc.tile_pool(name="ps_qk", bufs=3, space="PSUM") as qkps:
                for fi in range(4):
                    for tch in range(4):
                        ps = qkps.tile([128, 512], dt.float32)
                        for s in range(KS):
                            nc.tensor.matmul(
                                ps[:],
                                wqkT_sb[:, s, fi * 128 : (fi + 1) * 128],
                                xT_sb[:, s, tch * 512 : (tch + 1) * 512],
                                start=(s == 0),
                                stop=(s == KS - 1),
                            )
                        nc.vector.tensor_scalar_add(
                            qkT_sb[:, fi, tch * 512 : (tch + 1) * 512],
                            ps[:],
                            bqk_sb[:, fi : fi + 1],
                        )

            # ---- Phase 1b: v projection -> v_sb [t, (d|1)*3] (bf16, +bias) ----
            with tc.tile_pool(name="ps_v", bufs=3, space="PSUM") as vps:
                for ti in range(NT):
                    ps = vps.tile([128, FV], dt.float32)
                    for s in range(KS):
                        nc.tensor.matmul(
                            ps[:],
                            xT_sb[:, s, ti * 128 : (ti + 1) * 128],
                            wvT_sb[:, s, :],
                            start=(s == 0),
                            stop=(s == KS - 1),
                        )
                    nc.vector.tensor_tensor(
                        v_sb[:, ti, :], ps[:], bv_sb[:], mybir.AluOpType.add
                    )

            # ---- Phase 2: attention, Y^T = V^T P per 512-wide q chunk ----
            with (
                tc.tile_pool(name="ps_s", bufs=3, space="PSUM") as sps,
                tc.tile_pool(name="ps_y", bufs=2, space="PSUM") as yps,
                tc.tile_pool(name="ps_bc", bufs=1, space="PSUM") as bcp,
                tc.tile_pool(name="ps_o", bufs=1, space="PSUM") as ops_,
                tc.tile_pool(name="pt", bufs=3) as ptp,
                tc.tile_pool(name="rcl", bufs=2) as rcp_p,
                tc.tile_pool(name="bcs", bufs=2) as bcsp,
                tc.tile_pool(name="y1", bufs=2) as y1p,
                tc.tile_pool(name="ob", bufs=3) as obp,
            ):
                def hoff(h):
                    qf, kf = 64 * h, 256 + 64 * h
                    return qf // 128, qf % 128, kf // 128, kf % 128

                def emit_score(qc, h, j, sp):
                    q0 = qc * 512
                    qti, qoff, kti, koff = hoff(h)
                    w0 = max(j * 128, q0) if causal else q0
                    w = q0 + 512 - w0
                    nc.tensor.matmul(
                        sp[:, 0:w],
                        qkT_sb[koff : koff + 64, kti, j * 128 : (j + 1) * 128],
                        qkT_sb[qoff : qoff + 64, qti, w0 : q0 + 512],
                        start=True,
                        stop=True,
                    )
                    return w0, w

                def emit_epilogue(qc, h, ypt):
                    q0 = qc * 512
                    rc = rcp_p.tile([128, 512], dt.float16, name="rc")
                    with nc.allow_low_precision(reason="softmax denom rcp"):
                        nc.vector.reciprocal(rc[64:65, :], ypt[64:65, :])
                    bc = bcp.tile([64, 512], dt.float32, name="bc")
                    nc.tensor.matmul(
                        bc[:], ones16[64:65, :], rc[64:65, :], start=True, stop=True
                    )
                    bcs = bcsp.tile([128, 512], dt.bfloat16, name="bcs")
                    nc.vector.tensor_copy(bcs[0:64, :], bc[:])
                    if h == 1:
                        y1 = y1p.tile([128, 512], dt.bfloat16, name="y1")
                        nc.vector.tensor_tensor(
                            y1[0:64, :], ypt[0:64, :], bcs[0:64, :],
                            mybir.AluOpType.mult,
                        )
                        nc.sync.dma_start(
                            yT_sb[64:128, 0, q0 : q0 + 512], y1[0:64, :]
                        )
                    else:
                        hs = 0 if h == 0 else 1
                        nc.vector.tensor_tensor(
                            yT_sb[0:64, hs, q0 : q0 + 512],
                            ypt[0:64, :], bcs[0:64, :],
                            mybir.AluOpType.mult,
                        )

                def emit_outproj(qc):
                    for qt in range(4 * qc, 4 * qc + 4):
                        po1 = ops_.tile([128, 384], dt.float32, tag="po1")
                        po2 = ops_.tile([128, 384], dt.float32, tag="po2")
                        for lo, po in ((0, po1), (384, po2)):
                            nc.tensor.matmul(
                                po[:],
                                yT_sb[:, 0, qt * 128 : (qt + 1) * 128],
                                wpT_sb[:, 0, lo : lo + 384],
                                start=True,
                                stop=False,
                            )
                            nc.tensor.matmul(
                                po[:],
                                yT_sb[0:64, 1, qt * 128 : (qt + 1) * 128],
                                wpT_sb[0:64, 1, lo : lo + 384],
                                start=False,
                                stop=True,
                            )
                        ob = obp.tile([128, C], dt.float16)
                        nc.vector.tensor_copy(ob[:, 0:384], po1[:])
                        nc.vector.tensor_copy(ob[:, 384:768], po2[:])
                        nc.sync.dma_start(
                            out_d.ap()[qt * 128 : (qt + 1) * 128, :], ob[:]
                        )

                pending = None  # (qc, h, ypt) awaiting epilogue
                for qc in range(QC):
                    q0 = qc * 512
                    jtop = 4 * qc + 4 if causal else NT
                    for h in range(G):
                        ypt = yps.tile([128, 512], dt.float32, name="ypt")
                        sp0 = sps.tile([128, 512], dt.float32, name="sp")
                        meta0 = emit_score(qc, h, 0, sp0)
                        sls = [(sp0, meta0)]
                        if jtop > 1:
                            sp1 = sps.tile([128, 512], dt.float32, name="sp")
                            sls.append((sp1, emit_score(qc, h, 1, sp1)))
                        if pending is not None:
                            pq, phh, pypt = pending
                            emit_epilogue(pq, phh, pypt)
                            if phh == 2:
                                emit_outproj(pq)
                            pending = None
                        for j in range(jtop):
                            sp, (w0, w) = sls[j]
                            pt = ptp.tile([128, 512], dt.bfloat16, name="pt")
                            nc.scalar.activation(
                                pt[:, 0:w], sp[:, 0:w], Exp, scale=0.125
                            )
                            if causal and j * 128 >= q0:
                                nc.gpsimd.tensor_tensor(
                                    pt[:, 0:128], pt[:, 0:128], maskT_sb[:],
                                    mybir.AluOpType.mult,
                                )
                            if j + 2 < jtop:
                                spn = sps.tile([128, 512], dt.float32, name="sp")
                                sls.append((spn, emit_score(qc, h, j + 2, spn)))
                            nc.tensor.matmul(
                                ypt[0:65, w0 - q0 : 512],
                                v_sb[:, j, h * 65 : (h + 1) * 65],
                                pt[:, 0:w],
                                start=(j == 0),
                                stop=(j == jtop - 1),
                                skip_group_check=True,
                            )
                        pending = (qc, h, ypt)
                pq, phh, pypt = pending
                emit_epilogue(pq, phh, pypt)
                emit_outproj(pq)

    nc.compile()
    return nc


def _prep_in_maps(x, Wqkv, bqkv, Wproj):
    in_maps = []
    maskT = np.triu(np.ones((128, 128), dtype=np.float32)).astype(BF16)
    for c in range(8):
        b, hg = c // 4, c % 4
        r0 = 192 * hg
        xT = np.ascontiguousarray(x[b].T).astype(BF16)
        wqk = np.zeros((512, 768), dtype=np.float32)
        wqk[0:192] = Wqkv[r0 : r0 + 192]
        wqk[256:448] = Wqkv[768 + r0 : 768 + r0 + 192]
        wqkT = np.ascontiguousarray(wqk.T).astype(BF16)
        bqk_vec = np.zeros(512, dtype=np.float32)
        bqk_vec[0:192] = bqkv[r0 : r0 + 192]
        bqk_vec[256:448] = bqkv[768 + r0 : 768 + r0 + 192]
        bqk = np.ascontiguousarray(bqk_vec.reshape(4, 128).T.astype(np.float32))
        # augmented V: per head h cols h*65..h*65+63 = Wv^T slice, col h*65+64 = 0
        wvT = Wqkv[1536 + r0 : 1536 + r0 + 192].T.astype(np.float32)  # [768, 192]
        wv_aug = np.zeros((768, FV), dtype=np.float32)
        bv_aug = np.zeros(FV, dtype=np.float32)
        for h in range(G):
            wv_aug[:, h * 65 : h * 65 + 64] = wvT[:, h * 64 : (h + 1) * 64]
            bv_aug[h * 65 : h * 65 + 64] = bqkv[1536 + r0 + h * 64 : 1536 + r0 + (h + 1) * 64]
            bv_aug[h * 65 + 64] = 1.0
        bv = np.tile(bv_aug[None, :], (128, 1)).astype(np.float32)
        wp = np.zeros((256, 768), dtype=BF16)
        wp[0:192] = Wproj[:, r0 : r0 + 192].T.astype(BF16)
        in_maps.append(
            {
                "xT": xT,
                "wqkT": np.ascontiguousarray(wqkT),
                "wvT": np.ascontiguousarray(wv_aug.astype(BF16)),
                "bqk": np.ascontiguousarray(bqk),
                "bv": np.ascontiguousarray(bv),
                "wpT": wp,
                "maskT": maskT,
            }
        )
    return in_maps


def kernel(x, Wqkv, bqkv, Wproj, bproj, is_causal):
    global _last_in_maps
    x = np.asarray(x, dtype=np.float32)
    Wqkv = np.asarray(Wqkv, dtype=np.float32)
    bqkv = np.asarray(bqkv, dtype=np.float32)
    Wproj = np.asarray(Wproj, dtype=np.float32)
    bproj = np.asarray(bproj, dtype=np.float32)
    causal = bool(int(np.asarray(is_causal)))

    if causal not in _cache:
        _cache[causal] = _build(causal)
    nc = _cache[causal]

    in_maps = _prep_in_maps(x, Wqkv, bqkv, Wproj)
    _last_in_maps = in_maps
    res = run_bass_kernel_spmd(nc, in_maps, core_ids=list(range(8)))

    out = np.empty((B, T, C), dtype=np.float32)
    for b in range(B):
        acc = res.results[4 * b]["out"].astype(np.float32)
        for k in range(1, 4):
            acc += res.results[4 * b + k]["out"].astype(np.float32)
        out[b] = acc + bproj[None, :]
    return out


# revision 9
# speedup vs baseline: 1.1752x; 1.1752x over previous
"""Causal self-attention (B=2, T=2048, C=768, H=12) on 8 TRN2 NeuronCores.

Sharding: core c = (b = c // 4, head-group hg = c % 4 of 3 heads).
Each core: QKV projection for its 3 heads (column-parallel), causal
attention, and a row-parallel slice of the output projection. The host
pre-transposes/casts operands to bf16 and sums the 4 partial outputs
per batch (row-parallel all-reduce done host-side) + bias.

Attention uses the Y^T = V^T @ P formulation: scores are computed
transposed [k, q], exp'd, and one wide matmul per (j-block, head)
accumulates Y^T [d, q] in PSUM with the softmax denominator riding in
an extra ones column of V. Normalization = reciprocal of the denom row,
broadcast via a 1-contraction fp16 matmul, multiplied on DVE.
"""

import os
import sys

import numpy as np
import ml_dtypes


def _ensure_paths():
    for p in ("/opt/trn_rl_repo", "/opt/pypackages"):
        if os.path.isdir(p) and p not in sys.path:
            sys.path.append(p)


_ensure_paths()

import concourse.bass as bass  # noqa: E402
import concourse.mybir as mybir  # noqa: E402
import concourse.tile as tile  # noqa: E402
from concourse import bacc  # noqa: E402
from concourse.bass_utils import run_bass_kernel_spmd  # noqa: E402

BF16 = ml_dtypes.bfloat16
F16 = np.float16

B, T, C, H, D = 2, 2048, 768, 12, 64
G = 3                # heads per core
FQK = 512            # q(192) | pad(64) | k(192) | pad(64) -> q/k same partition offsets
FV = G * (D + 1)     # 195: per head [d0..d63 | ones]
NT = T // 128        # 16 token tiles
KS = C // 128        # 6 contraction subtiles
QC = NT // 4         # 4 q-chunks of 512

_cache: dict[bool, object] = {}
_last_in_maps = None


def _build(causal: bool):
    dt = mybir.dt
    nc = bacc.Bacc("TRN2", num_devices=8)

    xT_d = nc.dram_tensor("xT", [C, T], dt.bfloat16, kind="ExternalInput")
    wqkT_d = nc.dram_tensor("wqkT", [C, FQK], dt.bfloat16, kind="ExternalInput")
    wvT_d = nc.dram_tensor("wvT", [C, FV], dt.bfloat16, kind="ExternalInput")
    bqk_d = nc.dram_tensor("bqk", [128, 4], dt.float32, kind="ExternalInput")
    bv_d = nc.dram_tensor("bv", [128, FV], dt.float32, kind="ExternalInput")
    wpT_d = nc.dram_tensor("wpT", [256, C], dt.bfloat16, kind="ExternalInput")
    maskT_d = nc.dram_tensor("maskT", [128, 128], dt.bfloat16, kind="ExternalInput")
    out_d = nc.dram_tensor("out", [T, C], dt.float16, kind="ExternalOutput")

    Exp = mybir.ActivationFunctionType.Exp

    with tile.TileContext(nc) as tc:
        with tc.tile_pool(name="persist", bufs=1) as pp:
            xT_sb = pp.tile([128, KS, T], dt.bfloat16)
            wqkT_sb = pp.tile([128, KS, FQK], dt.bfloat16)
            wvT_sb = pp.tile([128, KS, FV], dt.bfloat16)
            wpT_sb = pp.tile([128, 2, C], dt.bfloat16)
            bqk_sb = pp.tile([128, 4], dt.float32)
            bv_sb = pp.tile([128, FV], dt.float32)
            maskT_sb = pp.tile([128, 128], dt.bfloat16)
            ones16 = pp.tile([128, 64], dt.float16)
            qkT_sb = pp.tile([128, 4, T], dt.bfloat16)
            v_sb = pp.tile([128, NT, FV], dt.bfloat16)
            yT_sb = pp.tile([128, 2, T], dt.bfloat16)

            # input DMAs issued from the Pool queue (cheap issue slot)
            nc.gpsimd.dma_start(maskT_sb[:], maskT_d.ap())
            nc.gpsimd.dma_start(bqk_sb[:], bqk_d.ap())
            nc.gpsimd.dma_start(bv_sb[:], bv_d.ap())
            for s in range(KS):
                nc.gpsimd.dma_start(
                    wqkT_sb[:, s, :], wqkT_d.ap()[s * 128 : (s + 1) * 128, :]
                )
                nc.gpsimd.dma_start(
                    xT_sb[:, s, :], xT_d.ap()[s * 128 : (s + 1) * 128, :]
                )
            nc.gpsimd.dma_start(
                wvT_sb[:], wvT_d.ap().rearrange("(s p) f -> p s f", p=128)
            )
            nc.gpsimd.dma_start(
                wpT_sb[:], wpT_d.ap().rearrange("(s p) o -> p s o", p=128)
            )
            nc.gpsimd.memset(ones16[:], 1.0)

            # ---- Phase 1a: q/k projection -> qkT_sb [f, t] (bf16, +bias) ----
            with tc.tile_pool(name="ps_qk", bufs=3, space="PSUM") as qkps:
                for fi in range(4):
                    for tch in range(4):
                        ps = qkps.tile([128, 512], dt.float32)
                        for s in range(KS):
                            nc.tensor.matmul(
                                ps[:],
                                wqkT_sb[:, s, fi * 128 : (fi + 1) * 128],
                                xT_sb[:, s, tch * 512 : (tch + 1) * 512],
                                start=(s == 0),
                                stop=(s == KS - 1),
                            )
                        nc.vector.tensor_scalar_add(
                            qkT_sb[:, fi, tch * 512 : (tch + 1) * 512],
                            ps[:],
                            bqk_sb[:, fi : fi + 1],
                        )

            # ---- Phase 1b: v projection -> v_sb [t, (d|1)*3] (bf16, +bias) ----
            with tc.tile_pool(name="ps_v", bufs=3, space="PSUM") as vps:
                for ti in range(NT):
                    ps = vps.tile([128, FV], dt.float32)
                    for s in range(KS):
                        nc.tensor.matmul(
                            ps[:],
                            xT_sb[:, s, ti * 128 : (ti + 1) * 128],
                            wvT_sb[:, s, :],
                            start=(s == 0),
                            stop=(s == KS - 1),
                        )
                    nc.vector.tensor_tensor(
                        v_sb[:, ti, :], ps[:], bv_sb[:], mybir.AluOpType.add
                    )

            # ---- Phase 2: attention, Y^T = V^T P per 512-wide q chunk ----
            # Pair-level software pipeline: scores+exp+mask of pair i
            # interleave (on the PE queue) with the attn@V accumulation of
            # pair i-1, whose pt tiles were produced a full pair earlier, so
            # every PE instruction has its inputs long ready. The softmax
            # denominator rides in the ones column of v_sb (psum row 64);
            # normalization broadcasts the denom row via a 1-contraction
            # fp16 matmul and divides on DVE.
            with (
                tc.tile_pool(name="ps_s", bufs=3, space="PSUM") as sps,
                tc.tile_pool(name="ps_y", bufs=2, space="PSUM") as yps,
                tc.tile_pool(name="ps_bc", bufs=1, space="PSUM") as bcp,
                tc.tile_pool(name="ps_o", bufs=1, space="PSUM") as ops_,
                tc.tile_pool(name="pt", bufs=20) as ptp,
                tc.tile_pool(name="rcl", bufs=2) as rcp_p,
                tc.tile_pool(name="bcs", bufs=2) as bcsp,
                tc.tile_pool(name="y1", bufs=2) as y1p,
                tc.tile_pool(name="ob", bufs=3) as obp,
            ):
                def hoff(h):
                    qf, kf = 64 * h, 256 + 64 * h
                    return qf // 128, qf % 128, kf // 128, kf % 128

                def emit_score_exp(qc, h, j):
                    """score matmul + exp (+ causal mask); returns pt meta."""
                    q0 = qc * 512
                    qti, qoff, kti, koff = hoff(h)
                    w0 = max(j * 128, q0) if causal else q0
                    w = q0 + 512 - w0
                    sp = sps.tile([128, 512], dt.float32, name="sp")
                    nc.tensor.matmul(
                        sp[:, 0:w],
                        qkT_sb[koff : koff + 64, kti, j * 128 : (j + 1) * 128],
                        qkT_sb[qoff : qoff + 64, qti, w0 : q0 + 512],
                        start=True,
                        stop=True,
                    )
                    pt = ptp.tile([128, 512], dt.bfloat16, name="pt")
                    nc.scalar.activation(pt[:, 0:w], sp[:, 0:w], Exp, scale=0.125)
                    if causal and j * 128 >= q0:
                        nc.gpsimd.tensor_tensor(
                            pt[:, 0:128], pt[:, 0:128], maskT_sb[:],
                            mybir.AluOpType.mult,
                        )
                    return pt, w0, w

                def emit_attnv(qc, h, j, jtop, ypt, meta):
                    q0 = qc * 512
                    pt, w0, w = meta
                    nc.tensor.matmul(
                        ypt[0:65, w0 - q0 : 512],
                        v_sb[:, j, h * 65 : (h + 1) * 65],
                        pt[:, 0:w],
                        start=(j == 0),
                        stop=(j == jtop - 1),
                        skip_group_check=True,
                    )

                def emit_epilogue(qc, h, ypt):
                    q0 = qc * 512
                    dn = rcp_p.tile([128, 512], dt.float16, name="dn")
                    nc.vector.tensor_copy(dn[64:65, :], ypt[64:65, :])
                    bc = bcp.tile([64, 512], dt.float32, name="bc")
                    nc.tensor.matmul(
                        bc[:], ones16[64:65, :], dn[64:65, :], start=True, stop=True
                    )
                    bcs = bcsp.tile([128, 512], dt.float32, name="bcs")
                    with nc.allow_low_precision(reason="softmax denom rcp"):
                        nc.vector.reciprocal_approx_fast(bcs[0:64, :], bc[:])
                    if h == 1:
                        y1 = y1p.tile([128, 512], dt.bfloat16, name="y1")
                        nc.vector.tensor_tensor(
                            y1[0:64, :], ypt[0:64, :], bcs[0:64, :],
                            mybir.AluOpType.mult,
                        )
                        nc.sync.dma_start(
                            yT_sb[64:128, 0, q0 : q0 + 512], y1[0:64, :]
                        )
                    else:
                        hs = 0 if h == 0 else 1
                        nc.vector.tensor_tensor(
                            yT_sb[0:64, hs, q0 : q0 + 512],
                            ypt[0:64, :], bcs[0:64, :],
                            mybir.AluOpType.mult,
                        )

                def emit_outproj(qc):
                    for qt in range(4 * qc, 4 * qc + 4):
                        po1 = ops_.tile([128, 384], dt.float32, tag="po1")
                        po2 = ops_.tile([128, 384], dt.float32, tag="po2")
                        for lo, po in ((0, po1), (384, po2)):
                            nc.tensor.matmul(
                                po[:],
                                yT_sb[:, 0, qt * 128 : (qt + 1) * 128],
                                wpT_sb[:, 0, lo : lo + 384],
                                start=True,
                                stop=False,
                            )
                            nc.tensor.matmul(
                                po[:],
                                yT_sb[0:64, 1, qt * 128 : (qt + 1) * 128],
                                wpT_sb[0:64, 1, lo : lo + 384],
                                start=False,
                                stop=True,
                            )
                        ob = obp.tile([128, C], dt.float16)
                        nc.vector.tensor_copy(ob[:, 0:384], po1[:])
                        nc.vector.tensor_copy(ob[:, 384:768], po2[:])
                        nc.sync.dma_start(
                            out_d.ap()[qt * 128 : (qt + 1) * 128, :], ob[:]
                        )

                pairs = [(qc, h) for qc in range(QC) for h in range(G)]

                def jtop_of(qc):
                    return 4 * qc + 4 if causal else NT

                prev = None  # (qc, h, jtop, ypt, metas)
                for qc, h in pairs + [(None, None)]:
                    jtop = jtop_of(qc) if qc is not None else 0
                    pjtop = prev[2] if prev is not None else 0
                    metas = []
                    pypt = None
                    for t in range(max(jtop, pjtop)):
                        if t < jtop:
                            metas.append(emit_score_exp(qc, h, t))
                        if prev is not None and t < pjtop:
                            if t == 0:
                                pypt = yps.tile(
                                    [128, 512], dt.float32, name="ypt"
                                )
                                prev = (*prev[:3], pypt, prev[4])
                            emit_attnv(
                                prev[0], prev[1], t, pjtop, pypt, prev[4][t]
                            )
                        if (
                            prev is not None
                            and t == pjtop - 1
                        ):
                            emit_epilogue(prev[0], prev[1], pypt)
                    if prev is not None and prev[1] == 2:
                        emit_outproj(prev[0])
                    prev = (qc, h, jtop, None, metas) if qc is not None else None

    nc.compile()
    return nc


def _prep_in_maps(x, Wqkv, bqkv, Wproj):
    in_maps = []
    maskT = np.triu(np.ones((128, 128), dtype=np.float32)).astype(BF16)
    for c in range(8):
        b, hg = c // 4, c % 4
        r0 = 192 * hg
        xT = np.ascontiguousarray(x[b].T).astype(BF16)
        wqk = np.zeros((512, 768), dtype=np.float32)
        wqk[0:192] = Wqkv[r0 : r0 + 192]
        wqk[256:448] = Wqkv[768 + r0 : 768 + r0 + 192]
        wqkT = np.ascontiguousarray(wqk.T).astype(BF16)
        bqk_vec = np.zeros(512, dtype=np.float32)
        bqk_vec[0:192] = bqkv[r0 : r0 + 192]
        bqk_vec[256:448] = bqkv[768 + r0 : 768 + r0 + 192]
        bqk = np.ascontiguousarray(bqk_vec.reshape(4, 128).T.astype(np.float32))
        # augmented V: per head h cols h*65..h*65+63 = Wv^T slice, col h*65+64 = 0
        wvT = Wqkv[1536 + r0 : 1536 + r0 + 192].T.astype(np.float32)  # [768, 192]
        wv_aug = np.zeros((768, FV), dtype=np.float32)
        bv_aug = np.zeros(FV, dtype=np.float32)
        for h in range(G):
            wv_aug[:, h * 65 : h * 65 + 64] = wvT[:, h * 64 : (h + 1) * 64]
            bv_aug[h * 65 : h * 65 + 64] = bqkv[1536 + r0 + h * 64 : 1536 + r0 + (h + 1) * 64]
            bv_aug[h * 65 + 64] = 1.0
        bv = np.tile(bv_aug[None, :], (128, 1)).astype(np.float32)
        wp = np.zeros((256, 768), dtype=BF16)
        wp[0:192] = Wproj[:, r0 : r0 + 192].T.astype(BF16)
        in_maps.append(
            {
                "xT": xT,
                "wqkT": np.ascontiguousarray(wqkT),
                "wvT": np.ascontiguousarray(wv_aug.astype(BF16)),
                "bqk": np.ascontiguousarray(bqk),
                "bv": np.ascontiguousarray(bv),
                "wpT": wp,
                "maskT": maskT,
            }
        )
    return in_maps


def kernel(x, Wqkv, bqkv, Wproj, bproj, is_causal):
    global _last_in_maps
    x = np.asarray(x, dtype=np.float32)
    Wqkv = np.asarray(Wqkv, dtype=np.float32)
    bqkv = np.asarray(bqkv, dtype=np.float32)
    Wproj = np.asarray(Wproj, dtype=np.float32)
    bproj = np.asarray(bproj, dtype=np.float32)
    causal = bool(int(np.asarray(is_causal)))

    if causal not in _cache:
        _cache[causal] = _build(causal)
    nc = _cache[causal]

    in_maps = _prep_in_maps(x, Wqkv, bqkv, Wproj)
    _last_in_maps = in_maps
    res = run_bass_kernel_spmd(nc, in_maps, core_ids=list(range(8)))

    out = np.empty((B, T, C), dtype=np.float32)
    for b in range(B):
        acc = res.results[4 * b]["out"].astype(np.float32)
        for k in range(1, 4):
            acc += res.results[4 * b + k]["out"].astype(np.float32)
        out[b] = acc + bproj[None, :]
    return out


# revision 10
# speedup vs baseline: 1.7039x; 1.4499x over previous
"""Causal self-attention (B=2, T=2048, C=768, H=12) on 8 TRN2 NeuronCores.

Sharding: core c = (b = c // 4, head-group hg = c % 4 of 3 heads).
Each core: QKV projection for its 3 heads (column-parallel), causal
attention, and a row-parallel slice of the output projection. The host
pre-transposes/casts operands to bf16 and sums the 4 partial outputs
per batch (row-parallel all-reduce done host-side) + bias.

Attention uses the Y^T = V^T @ P formulation: scores are computed
transposed [k, q], exp'd on the ACT engine, and one wide matmul per
(j-block, head) accumulates Y^T [d, q] in PSUM with the softmax
denominator riding in an extra ones column of V. Normalization
broadcasts the denom row via a 1-contraction fp16 matmul and applies
reciprocal_approx_fast + multiply on DVE.

The QKV/V projections are interleaved into the attention phase: causal
chunk qc only needs projections of token chunks <= qc, so projection
chains for chunk qc+1 fill PE idle slots while the ACT engine works
through chunk qc's softmax — keeping the PE busy enough that the HAM
clock gate stays at 8/8 (2.4 GHz) instead of the idle default 4/8.

qk layout [384 = q0|q1 , k0|k1 , q2|k2] keeps q/k partition offsets
aligned for heads 0/1; head 2's k is copied to a 4th tile via an
SBUF->SBUF DMA so its score matmul also sees matching offsets.
"""

import os
import sys

import numpy as np
import ml_dtypes


def _ensure_paths():
    for p in ("/opt/trn_rl_repo", "/opt/pypackages"):
        if os.path.isdir(p) and p not in sys.path:
            sys.path.append(p)


_ensure_paths()

import concourse.bass as bass  # noqa: E402
import concourse.mybir as mybir  # noqa: E402
import concourse.tile as tile  # noqa: E402
from concourse import bacc  # noqa: E402
from concourse.bass_utils import run_bass_kernel_spmd  # noqa: E402

BF16 = ml_dtypes.bfloat16
F16 = np.float16

B, T, C, H, D = 2, 2048, 768, 12, 64
G = 3                # heads per core
FQK = 384            # q0|q1 , k0|k1 , q2|k2 (64 cols each)
FV = G * (D + 1)     # 195: per head [d0..d63 | ones]
NT = T // 128        # 16 token tiles
KS = C // 128        # 6 contraction subtiles
QC = NT // 4         # 4 q-chunks of 512

_cache: dict[bool, object] = {}
_last_in_maps = None


def _build(causal: bool):
    dt = mybir.dt
    nc = bacc.Bacc("TRN2", num_devices=8)

    xT_d = nc.dram_tensor("xT", [C, T], dt.bfloat16, kind="ExternalInput")
    wqkT_d = nc.dram_tensor("wqkT", [C, FQK], dt.bfloat16, kind="ExternalInput")
    wvT_d = nc.dram_tensor("wvT", [C, FV], dt.bfloat16, kind="ExternalInput")
    bqk_d = nc.dram_tensor("bqk", [128, 3], dt.float32, kind="ExternalInput")
    bv_d = nc.dram_tensor("bv", [128, FV], dt.float32, kind="ExternalInput")
    wpT_d = nc.dram_tensor("wpT", [256, C], dt.bfloat16, kind="ExternalInput")
    maskT_d = nc.dram_tensor("maskT", [128, 128], dt.bfloat16, kind="ExternalInput")
    out_d = nc.dram_tensor("out", [T, C], dt.float16, kind="ExternalOutput")

    Exp = mybir.ActivationFunctionType.Exp

    with tile.TileContext(nc) as tc:
        with tc.tile_pool(name="persist", bufs=1) as pp:
            xT_sb = pp.tile([128, KS, T], dt.bfloat16)
            wqkT_sb = pp.tile([128, KS, FQK], dt.bfloat16)
            wvT_sb = pp.tile([128, KS, FV], dt.bfloat16)
            wpT_sb = pp.tile([128, 2, C], dt.bfloat16)
            bqk_sb = pp.tile([128, 3], dt.float32)
            bv_sb = pp.tile([128, FV], dt.float32)
            maskT_sb = pp.tile([128, 128], dt.bfloat16)
            ones16 = pp.tile([128, 64], dt.float16)
            qkT_sb = pp.tile([128, 4, T], dt.bfloat16)  # tile 3 = k2 copy
            v_sb = pp.tile([128, NT, FV], dt.bfloat16)
            yT_sb = pp.tile([128, 2, T], dt.bfloat16)

            # Input DMAs on the SP queue, ordered so the first qk chain's
            # deps (wqkT s*, xT s* first 512 tokens) land first.
            nc.sync.dma_start(maskT_sb[:], maskT_d.ap())
            nc.sync.dma_start(bqk_sb[:], bqk_d.ap())
            for s in range(KS):
                nc.sync.dma_start(
                    wqkT_sb[:, s, :], wqkT_d.ap()[s * 128 : (s + 1) * 128, :]
                )
                nc.sync.dma_start(
                    xT_sb[:, s, 0:512], xT_d.ap()[s * 128 : (s + 1) * 128, 0:512]
                )
            nc.sync.dma_start(bv_sb[:], bv_d.ap())
            nc.sync.dma_start(
                wvT_sb[:], wvT_d.ap().rearrange("(s p) f -> p s f", p=128)
            )
            for s in range(KS):
                nc.sync.dma_start(
                    xT_sb[:, s, 512:T], xT_d.ap()[s * 128 : (s + 1) * 128, 512:T]
                )
            nc.sync.dma_start(
                wpT_sb[:], wpT_d.ap().rearrange("(s p) o -> p s o", p=128)
            )
            nc.gpsimd.memset(ones16[:], 1.0)

            with (
                tc.tile_pool(name="ps_s", bufs=3, space="PSUM") as sps,
                tc.tile_pool(name="ps_y", bufs=2, space="PSUM") as yps,
                tc.tile_pool(name="ps_wk", bufs=3, space="PSUM") as wkp,
                tc.tile_pool(name="pt", bufs=20) as ptp,
                tc.tile_pool(name="bcs", bufs=2) as bcsp,
                tc.tile_pool(name="rcl", bufs=2) as rcp_p,
                tc.tile_pool(name="y1", bufs=2) as y1p,
                tc.tile_pool(name="ob", bufs=3) as obp,
            ):
                # ---- projection chains (interleaved into attention) ----
                def emit_qk_chain(tch, fi):
                    ps = wkp.tile([128, 512], dt.float32, name="wk")
                    for s in range(KS):
                        nc.tensor.matmul(
                            ps[:],
                            wqkT_sb[:, s, fi * 128 : (fi + 1) * 128],
                            xT_sb[:, s, tch * 512 : (tch + 1) * 512],
                            start=(s == 0),
                            stop=(s == KS - 1),
                        )
                    nc.vector.tensor_scalar_add(
                        qkT_sb[:, fi, tch * 512 : (tch + 1) * 512],
                        ps[:],
                        bqk_sb[:, fi : fi + 1],
                    )
                    if fi == 2:
                        # head-2 k lives at partition offset 64 of tile 2;
                        # shift to offset 0 of tile 3 so h2 scores see
                        # aligned q/k partition offsets.
                        nc.sync.dma_start(
                            qkT_sb[0:64, 3, tch * 512 : (tch + 1) * 512],
                            qkT_sb[64:128, 2, tch * 512 : (tch + 1) * 512],
                        )

                def emit_v_chain(tch, ti):
                    ps = wkp.tile([128, 512], dt.float32, name="wk")
                    for s in range(KS):
                        nc.tensor.matmul(
                            ps[:, 0:FV],
                            xT_sb[:, s, ti * 128 : (ti + 1) * 128],
                            wvT_sb[:, s, :],
                            start=(s == 0),
                            stop=(s == KS - 1),
                        )
                    nc.vector.tensor_tensor(
                        v_sb[:, ti, :], ps[:, 0:FV], bv_sb[:], mybir.AluOpType.add
                    )

                def chains_for(tch):
                    fns = []
                    for fi in range(3):
                        fns.append(lambda fi=fi: emit_qk_chain(tch, fi))
                    for ti in range(4 * tch, 4 * tch + 4):
                        fns.append(lambda ti=ti: emit_v_chain(tch, ti))
                    return fns

                # ---- attention emission helpers ----
                def hoff(h):
                    # (qti, qoff, kti, koff) with matching offsets per head
                    return [(0, 0, 1, 0), (0, 64, 1, 64), (2, 0, 3, 0)][h]

                def emit_score_exp(qc, h, j):
                    q0 = qc * 512
                    qti, qoff, kti, koff = hoff(h)
                    w0 = max(j * 128, q0) if causal else q0
                    w = q0 + 512 - w0
                    sp = sps.tile([128, 512], dt.float32, name="sp")
                    nc.tensor.matmul(
                        sp[:, 0:w],
                        qkT_sb[koff : koff + 64, kti, j * 128 : (j + 1) * 128],
                        qkT_sb[qoff : qoff + 64, qti, w0 : q0 + 512],
                        start=True,
                        stop=True,
                    )
                    pt = ptp.tile([128, 512], dt.bfloat16, name="pt")
                    nc.scalar.activation(pt[:, 0:w], sp[:, 0:w], Exp, scale=0.125)
                    if causal and j * 128 >= q0:
                        nc.gpsimd.tensor_tensor(
                            pt[:, 0:128], pt[:, 0:128], maskT_sb[:],
                            mybir.AluOpType.mult,
                        )
                    return pt, w0, w

                def emit_attnv(qc, h, j, jtop, ypt, meta):
                    q0 = qc * 512
                    pt, w0, w = meta
                    nc.tensor.matmul(
                        ypt[0:65, w0 - q0 : 512],
                        v_sb[:, j, h * 65 : (h + 1) * 65],
                        pt[:, 0:w],
                        start=(j == 0),
                        stop=(j == jtop - 1),
                        skip_group_check=True,
                    )

                def emit_epilogue(qc, h, ypt):
                    q0 = qc * 512
                    dn = rcp_p.tile([128, 512], dt.float16, name="dn")
                    nc.vector.tensor_copy(dn[64:65, :], ypt[64:65, :])
                    bc = wkp.tile([128, 512], dt.float32, name="wk")
                    nc.tensor.matmul(
                        bc[0:64, :], ones16[64:65, :], dn[64:65, :],
                        start=True, stop=True,
                    )
                    bcs = bcsp.tile([128, 512], dt.float32, name="bcs")
                    with nc.allow_low_precision(reason="softmax denom rcp"):
                        nc.vector.reciprocal_approx_fast(bcs[0:64, :], bc[0:64, :])
                    if h == 1:
                        y1 = y1p.tile([128, 512], dt.bfloat16, name="y1")
                        nc.vector.tensor_tensor(
                            y1[0:64, :], ypt[0:64, :], bcs[0:64, :],
                            mybir.AluOpType.mult,
                        )
                        nc.sync.dma_start(
                            yT_sb[64:128, 0, q0 : q0 + 512], y1[0:64, :]
                        )
                    else:
                        hs = 0 if h == 0 else 1
                        nc.vector.tensor_tensor(
                            yT_sb[0:64, hs, q0 : q0 + 512],
                            ypt[0:64, :], bcs[0:64, :],
                            mybir.AluOpType.mult,
                        )

                def emit_outproj(qc):
                    for qt in range(4 * qc, 4 * qc + 4):
                        po1 = wkp.tile([128, 512], dt.float32, name="wk")
                        po2 = wkp.tile([128, 512], dt.float32, name="wk")
                        for lo, po in ((0, po1), (384, po2)):
                            nc.tensor.matmul(
                                po[:, 0:384],
                                yT_sb[:, 0, qt * 128 : (qt + 1) * 128],
                                wpT_sb[:, 0, lo : lo + 384],
                                start=True,
                                stop=False,
                            )
                            nc.tensor.matmul(
                                po[:, 0:384],
                                yT_sb[0:64, 1, qt * 128 : (qt + 1) * 128],
                                wpT_sb[0:64, 1, lo : lo + 384],
                                start=False,
                                stop=True,
                            )
                        ob = obp.tile([128, C], dt.float16)
                        nc.vector.tensor_copy(ob[:, 0:384], po1[:, 0:384])
                        nc.vector.tensor_copy(ob[:, 384:768], po2[:, 0:384])
                        nc.sync.dma_start(
                            out_d.ap()[qt * 128 : (qt + 1) * 128, :], ob[:]
                        )

                def jtop_of(qc):
                    return 4 * qc + 4 if causal else NT

                # ---- integrated schedule ----
                for fn in chains_for(0):
                    fn()

                pairs = [(qc, h) for qc in range(QC) for h in range(G)]
                prev = None  # (qc, h, jtop, ypt, metas)
                pend_chains: list = []
                for qc, h in pairs + [(None, None)]:
                    if qc is not None and h == 0 and qc + 1 < QC:
                        pend_chains = chains_for(qc + 1)
                    jtop = jtop_of(qc) if qc is not None else 0
                    pjtop = prev[2] if prev is not None else 0
                    nsteps = max(jtop, pjtop)
                    # spread pending proj chains over this chunk's pairs
                    nchains = 0
                    if qc is not None and pend_chains:
                        rem_pairs = G - h
                        nchains = -(-len(pend_chains) // rem_pairs)
                    slots = set()
                    if nchains:
                        stride = max(1, nsteps // nchains)
                        slots = {k * stride for k in range(nchains)}
                    metas = []
                    pypt = None
                    for t in range(nsteps):
                        if t < jtop:
                            metas.append(emit_score_exp(qc, h, t))
                        if prev is not None and t < pjtop:
                            if t == 0:
                                pypt = yps.tile(
                                    [128, 512], dt.float32, name="ypt"
                                )
                            emit_attnv(
                                prev[0], prev[1], t, pjtop, pypt, prev[4][t]
                            )
                        if t in slots and pend_chains:
                            pend_chains.pop(0)()
                        if prev is not None and t == pjtop - 1:
                            emit_epilogue(prev[0], prev[1], pypt)
                    while qc is not None and h == 2 and pend_chains:
                        pend_chains.pop(0)()
                    if prev is not None and prev[1] == 2:
                        emit_outproj(prev[0])
                    prev = (qc, h, jtop, None, metas) if qc is not None else None

    nc.compile()
    return nc


def _prep_in_maps(x, Wqkv, bqkv, Wproj):
    in_maps = []
    maskT = np.triu(np.ones((128, 128), dtype=np.float32)).astype(BF16)
    for c in range(8):
        b, hg = c // 4, c % 4
        r0 = 192 * hg
        xT = np.ascontiguousarray(x[b].T).astype(BF16)
        Wq = Wqkv[r0 : r0 + 192]
        Wk = Wqkv[768 + r0 : 768 + r0 + 192]
        bq = bqkv[r0 : r0 + 192]
        bk = bqkv[768 + r0 : 768 + r0 + 192]
        # layout [q0|q1, k0|k1, q2|k2] (64-row groups)
        wqk = np.concatenate(
            [Wq[0:128], Wk[0:128], Wq[128:192], Wk[128:192]], axis=0
        )  # [384, 768]
        wqkT = np.ascontiguousarray(wqk.T.astype(BF16))
        bqk_vec = np.concatenate([bq[0:128], bk[0:128], bq[128:192], bk[128:192]])
        bqk = np.ascontiguousarray(bqk_vec.reshape(3, 128).T.astype(np.float32))
        # augmented V: per head h cols h*65..h*65+63 = Wv^T slice, col h*65+64 = 0
        wvT = Wqkv[1536 + r0 : 1536 + r0 + 192].T.astype(np.float32)  # [768, 192]
        wv_aug = np.zeros((768, FV), dtype=np.float32)
        bv_aug = np.zeros(FV, dtype=np.float32)
        for h in range(G):
            wv_aug[:, h * 65 : h * 65 + 64] = wvT[:, h * 64 : (h + 1) * 64]
            bv_aug[h * 65 : h * 65 + 64] = bqkv[
                1536 + r0 + h * 64 : 1536 + r0 + (h + 1) * 64
            ]
            bv_aug[h * 65 + 64] = 1.0
        bv = np.tile(bv_aug[None, :], (128, 1)).astype(np.float32)
        wp = np.zeros((256, 768), dtype=BF16)
        wp[0:192] = Wproj[:, r0 : r0 + 192].T.astype(BF16)
        in_maps.append(
            {
                "xT": xT,
                "wqkT": wqkT,
                "wvT": np.ascontiguousarray(wv_aug.astype(BF16)),
                "bqk": bqk,
                "bv": np.ascontiguousarray(bv),
                "wpT": wp,
                "maskT": maskT,
            }
        )
    return in_maps


def kernel(x, Wqkv, bqkv, Wproj, bproj, is_causal):
    global _last_in_maps
    x = np.asarray(x, dtype=np.float32)
    Wqkv = np.asarray(Wqkv, dtype=np.float32)
    bqkv = np.asarray(bqkv, dtype=np.float32)
    Wproj = np.asarray(Wproj, dtype=np.float32)
    bproj = np.asarray(bproj, dtype=np.float32)
    causal = bool(int(np.asarray(is_causal)))

    if causal not in _cache:
        _cache[causal] = _build(causal)
    nc = _cache[causal]

    in_maps = _prep_in_maps(x, Wqkv, bqkv, Wproj)
    _last_in_maps = in_maps
    res = run_bass_kernel_spmd(nc, in_maps, core_ids=list(range(8)))

    out = np.empty((B, T, C), dtype=np.float32)
    for b in range(B):
        acc = res.results[4 * b]["out"].astype(np.float32)
        for k in range(1, 4):
            acc += res.results[4 * b + k]["out"].astype(np.float32)
        out[b] = acc + bproj[None, :]
    return out
